# revision 9
# baseline (speedup 1.0000x reference)
"""Trainium2 Bass kernel for causal multi-head attention with RoPE.

Full-input contract: kernel(**inputs) takes the unsharded tensors and
returns the full [B, S, D] output. Internally the work is sharded over
8 NeuronCores: cores 0-3 compute batch 0, cores 4-7 batch 1; within a
batch group each core owns 4 of the 16 heads (tensor-parallel over
heads). Each core computes its partial output-projection contribution
[S, D]; the host sums the 4 partials per batch and adds the biases
that commute with attention (wo_b, and wv_b which passes through the
softmax untouched because attention weights sum to 1).

All matmul operands are bf16 (same 1 cycle/row PE rate as fp32r but
half the SBUF/DMA footprint and no small-free-dim penalty), with fp32
PSUM accumulation. The host converts inputs to bf16 before upload.

Every intermediate stays in SBUF. The three stages are interleaved per
512-query chunk n: project Q/K/V for chunk n (+RoPE), run causal
attention for query chunk n against keys 0..n, then emit the output
projection for chunk n-1. Softmax denominators are accumulated on the
DVE in bf16 and collapsed across partitions with a single 128x128 ones
matmul per (chunk, head) instead of a full row-sum matmul per score
tile, keeping that work off the PE.
"""

import os
import sys

sys.path.insert(0, "/opt/trn_rl_repo")

import numpy as np
import ml_dtypes

B = 2
S = 2048
D = 2048
H = 16
DK = 128
N_CORES = 8
HPC = 4          # heads per core
E = HPC * DK     # 512: per-core slice of the model dim
CH = 512         # sequence chunk (query chunk = projection chunk)
NCH = S // CH    # 4 chunks
KO = D // 128    # contraction chunks for the projections
NJ = S // 128    # key tiles
ISQRT_DK = 1.0 / np.sqrt(DK)

_CACHE = {}

last_exec_time_ns = None
last_results = None


def _build_program():
    import concourse.mybir as mybir
    import concourse.tile as tile
    from concourse import bacc

    dt = mybir.dt
    F32 = dt.float32
    BF16 = dt.bfloat16
    AF = mybir.ActivationFunctionType

    nc = bacc.Bacc(None, target_bir_lowering=False, debug=True)

    xT = nc.dram_tensor("xT", [D, S], BF16, kind="ExternalInput")
    wqT = nc.dram_tensor("wqT", [D, E], BF16, kind="ExternalInput")
    wkT = nc.dram_tensor("wkT", [D, E], BF16, kind="ExternalInput")
    wvT = nc.dram_tensor("wvT", [D, E], BF16, kind="ExternalInput")
    woT = nc.dram_tensor("woT", [E, D], BF16, kind="ExternalInput")
    bq = nc.dram_tensor("bq", [HPC, DK], F32, kind="ExternalInput")
    bk = nc.dram_tensor("bk", [HPC, DK], F32, kind="ExternalInput")
    cc2 = nc.dram_tensor("cc2", [DK, S], BF16, kind="ExternalInput")
    sss = nc.dram_tensor("sss", [DK, S], BF16, kind="ExternalInput")
    mask = nc.dram_tensor("mask", [128, 128], BF16, kind="ExternalInput")
    ones = nc.dram_tensor("ones", [128, 128], BF16, kind="ExternalInput")
    out = nc.dram_tensor("out", [S, D], BF16, kind="ExternalOutput")

    with tile.TileContext(nc) as tc:
        with (
            tc.tile_pool(name="const", bufs=1) as cpool,
            tc.tile_pool(name="w", bufs=1) as wpool,
            tc.tile_pool(name="x", bufs=2) as xpool,
            tc.tile_pool(name="kres", bufs=1) as krpool,
            tc.tile_pool(name="vres", bufs=1) as vpool,
            tc.tile_pool(name="q", bufs=2) as qpool,
            tc.tile_pool(name="rope", bufs=3) as rpool,
            tc.tile_pool(name="p", bufs=4) as ppool,
            tc.tile_pool(name="pl", bufs=2) as plpool,
            tc.tile_pool(name="li", bufs=2) as lipool,
            tc.tile_pool(name="ao", bufs=2) as aopool,
            tc.tile_pool(name="ob", bufs=4) as obpool,
            tc.tile_pool(name="psum", bufs=1, space="PSUM") as pspool,
        ):
            # ---- constants (small; off the critical DMA path) ----
            bq_sb = cpool.tile([DK, HPC], F32, name="bq_sb")
            nc.gpsimd.dma_start(bq_sb[:], bq[:].rearrange("h d -> d h"))
            bk_sb = cpool.tile([DK, HPC], F32, name="bk_sb")
            nc.gpsimd.dma_start(bk_sb[:], bk[:].rearrange("h d -> d h"))
            cc2_sb = cpool.tile([DK, S], BF16, name="cc2_sb")
            nc.gpsimd.dma_start(cc2_sb[:], cc2[:])
            sss_sb = cpool.tile([DK, S], BF16, name="sss_sb")
            nc.gpsimd.dma_start(sss_sb[:], sss[:])
            mask_sb = cpool.tile([128, 128], BF16, name="mask_sb")
            nc.gpsimd.dma_start(mask_sb[:], mask[:])
            ones_sb = cpool.tile([128, 128], BF16, name="ones_sb")
            nc.gpsimd.dma_start(ones_sb[:], ones[:])

            # ---- bulk weights/x: need-ordered stream ----
            # first Q matmul chain consumes (wq[k], x0[k]) pairs in k order,
            # so interleave those at 512-row granularity; wk/wv/wo behind.
            wq_sb = wpool.tile([128, KO, E], BF16, name="wq_sb")
            wk_sb = wpool.tile([128, KO, E], BF16, name="wk_sb")
            wv_sb = wpool.tile([128, KO, E], BF16, name="wv_sb")
            wo_sb = wpool.tile([128, HPC, D], BF16, name="wo_sb")

            def load_xn(n):
                xn = xpool.tile([128, KO, CH], BF16, tag="xn", name=f"xn{n}")
                for g in range(4):
                    nc.sync.dma_start(
                        xn[:, g * 4 : (g + 1) * 4, :],
                        xT[
                            g * 512 : (g + 1) * 512,
                            n * CH : (n + 1) * CH,
                        ].rearrange("(ko p) s -> p ko s", p=128),
                    )
                return xn

            xn_next = xpool.tile([128, KO, CH], BF16, tag="xn", name="xn0")
            for k in range(KO):
                nc.sync.dma_start(
                    wq_sb[:, k, :], wqT[k * 128 : (k + 1) * 128, :]
                )
                nc.sync.dma_start(
                    xn_next[:, k, :], xT[k * 128 : (k + 1) * 128, 0:CH]
                )
            for wsb, wdram in ((wk_sb, wkT), (wv_sb, wvT)):
                for k in range(KO):
                    nc.sync.dma_start(
                        wsb[:, k, :], wdram[k * 128 : (k + 1) * 128, :]
                    )
            for ec in range(HPC):
                nc.sync.dma_start(
                    wo_sb[:, ec, :], woT[ec * 128 : (ec + 1) * 128, :]
                )

            kr = krpool.tile([128, HPC, S], BF16, name="kr")
            vres = vpool.tile([128, NJ, E], BF16, name="vres")

            def emit_c(ao_t, n_src, ii):
                # output projection for rows [n_src*CH + ii*128, +128)
                r0 = n_src * CH + ii * 128
                for fc in range(4):
                    pc = pspool.tile([128, 512], F32, tag="aps", bufs=3)
                    for ec in range(HPC):
                        nc.tensor.matmul(
                            pc[:],
                            ao_t[:, ec, ii * 128 : (ii + 1) * 128],
                            wo_sb[:, ec, fc * 512 : (fc + 1) * 512],
                            start=(ec == 0),
                            stop=(ec == HPC - 1),
                        )
                    ob = obpool.tile([128, 512], BF16, tag="ob")
                    if fc % 2 == 0:
                        nc.vector.tensor_copy(ob[:], pc[:])
                    else:
                        nc.scalar.activation(ob[:], pc[:], AF.Copy)
                    nc.scalar.dma_start(
                        out[r0 : r0 + 128, fc * 512 : (fc + 1) * 512], ob[:]
                    )

            ao_prev = None
            for n in range(NCH):
                nsl = slice(n * CH, (n + 1) * CH)
                xn = xn_next
                if n + 1 < NCH:
                    xn_next = load_xn(n + 1)

                # ---- stage A: project chunk n (+RoPE on Q/K) ----
                qc = qpool.tile([128, HPC, CH], BF16, tag="qc", name=f"qc{n}")

                def rope(pq, bsb, dst):
                    st0 = rpool.tile([128, CH], BF16, tag="st0")
                    nc.scalar.activation(
                        st0[:], pq[:], AF.Identity, bias=bsb
                    )
                    # RoPE: d-rows packed [even; odd] per head, so the
                    # rotate pair is partition r <-> r+64
                    sw = rpool.tile([128, CH], BF16, tag="sw")
                    nc.vector.tensor_copy(sw[0:64, :], st0[64:128, :])
                    nc.vector.tensor_copy(sw[64:128, :], st0[0:64, :])
                    rot = rpool.tile([128, CH], BF16, tag="rot")
                    nc.vector.tensor_mul(rot[:], st0[:], cc2_sb[:, nsl])
                    nc.vector.tensor_mul(sw[:], sw[:], sss_sb[:, nsl])
                    nc.vector.tensor_add(dst, rot[:], sw[:])

                def psum4():
                    # four simultaneously-live psum tiles borrowed from the
                    # aps(2) + ps(3) rings for chunk-0 k-outer chains
                    return [
                        pspool.tile(
                            [128, CH],
                            F32,
                            tag=("aps" if i < 2 else "ps"),
                            bufs=(3 if i < 2 else 3),
                            name=f"pk0_{i}",
                        )
                        for i in range(HPC)
                    ]

                if n == 0:
                    # k-outer on the first chunk: the PE consumes each
                    # (weight, x) 512KB DMA group as it lands instead of
                    # stalling on the full tensors
                    for wsb, bsb, is_q in (
                        (wq_sb, bq_sb, True),
                        (wk_sb, bk_sb, False),
                    ):
                        pqs = psum4()
                        for k in range(KO):
                            for m in range(HPC):
                                nc.tensor.matmul(
                                    pqs[m][:],
                                    wsb[:, k, m * DK : (m + 1) * DK],
                                    xn[:, k, :],
                                    start=(k == 0),
                                    stop=(k == KO - 1),
                                )
                        for m in range(HPC):
                            dst = qc[:, m, :] if is_q else kr[:, m, nsl]
                            rope(pqs[m], bsb[:, m : m + 1], dst)
                    pvs = psum4()
                    for k in range(KO):
                        for jj in range(CH // 128):
                            nc.tensor.matmul(
                                pvs[jj][:],
                                xn[:, k, jj * 128 : (jj + 1) * 128],
                                wv_sb[:, k, :],
                                start=(k == 0),
                                stop=(k == KO - 1),
                            )
                    for jj in range(CH // 128):
                        nc.scalar.activation(
                            vres[:, n * 4 + jj, :], pvs[jj][:], AF.Copy
                        )
                else:
                    for wsb, bsb, is_q in (
                        (wq_sb, bq_sb, True),
                        (wk_sb, bk_sb, False),
                    ):
                        for m in range(HPC):
                            pq = pspool.tile([128, CH], F32, tag="aps", bufs=3)
                            for k in range(KO):
                                nc.tensor.matmul(
                                    pq[:],
                                    wsb[:, k, m * DK : (m + 1) * DK],
                                    xn[:, k, :],
                                    start=(k == 0),
                                    stop=(k == KO - 1),
                                )
                            dst = qc[:, m, :] if is_q else kr[:, m, nsl]
                            rope(pq, bsb[:, m : m + 1], dst)
                    for jj in range(CH // 128):
                        pvp = pspool.tile([128, E], F32, tag="aps", bufs=3)
                        for k in range(KO):
                            nc.tensor.matmul(
                                pvp[:],
                                xn[:, k, jj * 128 : (jj + 1) * 128],
                                wv_sb[:, k, :],
                                start=(k == 0),
                                stop=(k == KO - 1),
                            )
                        nc.scalar.activation(
                            vres[:, n * 4 + jj, :], pvp[:], AF.Copy
                        )

                # ---- stage B: attention for query chunk n ----
                # ---- stage C (interleaved): out-proj for chunk n-1 ----
                ao_cur = aopool.tile(
                    [128, HPC, CH], BF16, tag="ao", name=f"ao{n}"
                )
                # query windows: full chunk normally; the last chunk splits
                # in half so its output projection overlaps attention
                # instead of running as a serial tail
                if n < NCH - 1:
                    wins = [(0, CH)]
                else:
                    wins = [(0, CH // 2), (CH // 2, CH // 2)]
                passes = [(q0, qn, m) for (q0, qn) in wins for m in range(HPC)]

                def pass_njc(q0, qn):
                    return 4 * n + (q0 + qn) // 128

                def score_exp(m, jc, q0, qn):
                    t = jc - (4 * n + q0 // 128)  # >=0 on the diagonal band
                    cs = 128 * t if t >= 0 else 0
                    ps = pspool.tile([128, CH], F32, tag="ps", bufs=3)
                    nc.tensor.matmul(
                        ps[:, cs:qn],
                        kr[:, m, jc * 128 : (jc + 1) * 128],
                        qc[:, m, q0 + cs : q0 + qn],
                        start=True,
                        stop=True,
                    )
                    p = ppool.tile([128, CH], BF16, tag="p")
                    nc.scalar.activation(
                        p[:, cs:qn], ps[:, cs:qn], AF.Exp,
                        scale=float(ISQRT_DK),
                    )
                    if t >= 0:
                        nc.vector.tensor_mul(
                            p[:, cs : cs + 128],
                            p[:, cs : cs + 128],
                            mask_sb[:],
                        )
                    return (p, jc, cs)

                # software pipeline: scores run up to three tiles ahead of
                # the P@V matmuls (and are pre-warmed across head
                # boundaries) so the ACT exp latency stays off the
                # tensor-engine path
                warm = []
                for pi, (q0, qn, m) in enumerate(passes):
                    njc = pass_njc(q0, qn)
                    po = pspool.tile([128, CH], F32, tag="po", bufs=2)
                    pl = plpool.tile([128, CH], BF16, tag="pl")

                    def emit_pv(p, jc, cs):
                        # denominator accumulate (DVE, bf16) + P@V (PE)
                        if jc == 0:
                            nc.vector.tensor_copy(pl[:, cs:qn], p[:, cs:qn])
                        else:
                            nc.vector.tensor_add(
                                pl[:, cs:qn], pl[:, cs:qn], p[:, cs:qn]
                            )
                        nc.tensor.matmul(
                            po[:, cs:qn],
                            vres[:, jc, m * DK : (m + 1) * DK],
                            p[:, cs:qn],
                            start=(jc == 0),
                            stop=(jc == njc - 1),
                        )

                    pending = warm
                    warm = []
                    for jc in range(len(pending), njc):
                        pending.append(score_exp(m, jc, q0, qn))
                        if len(pending) > 2:
                            emit_pv(*pending.pop(0))
                    for it in pending:
                        emit_pv(*it)

                    # collapse the 128 partial-denominator rows with one
                    # 128x128 ones matmul (borrowing a "ps" ring slot),
                    # then normalize
                    pstot = pspool.tile([128, CH], F32, tag="ps", bufs=3)
                    nc.tensor.matmul(
                        pstot[:, :qn], ones_sb[:], pl[:, :qn],
                        start=True, stop=True,
                    )
                    li = lipool.tile([128, CH], F32, tag="li")
                    nc.vector.reciprocal_approx_fast(li[:, :qn], pstot[:, :qn])

                    # pre-warm the next pass's first score tiles while the
                    # epilogue and out-projection run
                    if pi + 1 < len(passes):
                        nq0, nqn, nm = passes[pi + 1]
                        for jc in range(min(3, pass_njc(nq0, nqn))):
                            warm.append(score_exp(nm, jc, nq0, nqn))

                    nc.vector.tensor_mul(
                        ao_cur[:, m, q0 : q0 + qn], po[:, :qn], li[:, :qn]
                    )

                    if q0 == 0:
                        if ao_prev is not None:
                            emit_c(ao_prev, n - 1, m)
                    elif m < 2:
                        # second window of the last chunk: rows of the first
                        # window are complete for all heads
                        emit_c(ao_cur, n, m)

                ao_prev = ao_cur

            emit_c(ao_prev, NCH - 1, 2)
            emit_c(ao_prev, NCH - 1, 3)

    nc.compile()
    return nc


def _rope_tables():
    inv_freq = 1.0 / (10000.0 ** (np.arange(0, DK, 2, dtype=np.float64) / DK))
    pos = np.arange(S, dtype=np.float64)
    freqs = pos[:, None] * inv_freq[None, :]  # [S, DK/2]
    cos_t = np.cos(freqs).T.astype(np.float32)  # [64, S]
    sin_t = np.sin(freqs).T.astype(np.float32)
    cc2 = np.ascontiguousarray(np.concatenate([cos_t, cos_t], axis=0))
    sss = np.ascontiguousarray(np.concatenate([-sin_t, sin_t], axis=0))
    return cc2, sss


def _bf16(a):
    return np.ascontiguousarray(a.astype(ml_dtypes.bfloat16))


def kernel(
    x, wq_w, wq_b, wk_w, wk_b, wv_w, wv_b, wo_w, wo_b
) -> np.ndarray:
    global last_exec_time_ns, last_results
    from concourse.bass_utils import run_bass_kernel_spmd

    if "nc" not in _CACHE:
        _CACHE["nc"] = _build_program()
    nc = _CACHE["nc"]

    x = np.asarray(x, dtype=np.float32)
    wq_w = np.asarray(wq_w, dtype=np.float32)
    wk_w = np.asarray(wk_w, dtype=np.float32)
    wv_w = np.asarray(wv_w, dtype=np.float32)
    wo_w = np.asarray(wo_w, dtype=np.float32)
    wq_b = np.asarray(wq_b, dtype=np.float32)
    wk_b = np.asarray(wk_b, dtype=np.float32)
    wv_b = np.asarray(wv_b, dtype=np.float32)
    wo_b = np.asarray(wo_b, dtype=np.float32)

    cc2, sss = _rope_tables()
    r_idx = np.arange(128)[:, None]
    c_idx = np.arange(128)[None, :]
    mask_np = (r_idx <= c_idx).astype(np.float32)
    ones = np.ones((128, 128), dtype=np.float32)
    # within each head, pack d-rows as [even dims; odd dims]
    perm = np.concatenate([np.arange(0, DK, 2), np.arange(1, DK, 2)])

    xT_b = [_bf16(x[b].T) for b in range(B)]

    in_maps = []
    for c in range(N_CORES):
        b = c // (N_CORES // B)
        g = c % (N_CORES // B)
        es = g * E

        def pack_qk(w):
            rows = w[es : es + E]  # [E, D]
            blocks = [
                rows[h0 * DK : (h0 + 1) * DK][perm] for h0 in range(HPC)
            ]
            return _bf16(np.concatenate(blocks, axis=0).T)

        def pack_bias(bvec):
            sl = bvec[es : es + E].reshape(HPC, DK)
            return np.ascontiguousarray(sl[:, perm])

        in_maps.append(
            {
                "xT": xT_b[b],
                "wqT": pack_qk(wq_w),
                "wkT": pack_qk(wk_w),
                "wvT": _bf16(wv_w[es : es + E].T),
                "woT": _bf16(wo_w[:, es : es + E].T),
                "bq": pack_bias(wq_b),
                "bk": pack_bias(wk_b),
                "cc2": _bf16(cc2),
                "sss": _bf16(sss),
                "mask": _bf16(mask_np),
                "ones": _bf16(ones),
            }
        )

    trace = bool(os.environ.get("MHA_TRACE"))
    res = run_bass_kernel_spmd(
        nc, in_maps, list(range(N_CORES)), trace=trace
    )
    last_exec_time_ns = res.exec_time_ns
    last_results = res

    # host-side gather: sum partials per batch, add biases that commute
    # with attention (softmax rows sum to 1, so wv_b passes straight
    # through to the output projection)
    const_bias = wo_b + wo_w @ wv_b  # [D]
    out = np.empty((B, S, D), dtype=np.float32)
    gpb = N_CORES // B
    for b in range(B):
        acc = res.results[b * gpb]["out"].astype(np.float32)
        for c in range(b * gpb + 1, (b + 1) * gpb):
            acc += res.results[c]["out"].astype(np.float32)
        out[b] = acc + const_bias[None, :]
    return out


# revision 12
# speedup vs baseline: 1.0636x; 1.0636x over previous
"""Trainium2 Bass kernel for causal multi-head attention with RoPE.

Full-input contract: kernel(**inputs) takes the unsharded tensors and
returns the full [B, S, D] output. Internally the work is sharded over
8 NeuronCores: cores 0-3 compute batch 0, cores 4-7 batch 1; within a
batch group each core owns 4 of the 16 heads (tensor-parallel over
heads). Each core computes its partial output-projection contribution
[S, D]; the host sums the 4 partials per batch and adds the biases
that commute with attention (wo_b, and wv_b which passes through the
softmax untouched because attention weights sum to 1).

All matmul operands are bf16 (same 1 cycle/row PE rate as fp32r but
half the SBUF/DMA footprint and no small-free-dim penalty), with fp32
PSUM accumulation. The host converts inputs to bf16 before upload.

Every intermediate stays in SBUF. The three stages are interleaved per
512-query chunk n: project Q/K/V for chunk n (+RoPE), run causal
attention for query chunk n against keys 0..n, then emit the output
projection for chunk n-1. Softmax denominators are accumulated on the
DVE in bf16 and collapsed across partitions with a single 128x128 ones
matmul per (chunk, head) instead of a full row-sum matmul per score
tile, keeping that work off the PE.
"""

import os
import sys

sys.path.insert(0, "/opt/trn_rl_repo")

import numpy as np
import ml_dtypes

B = 2
S = 2048
D = 2048
H = 16
DK = 128
N_CORES = 8
HPC = 4          # heads per core
E = HPC * DK     # 512: per-core slice of the model dim
CH = 512         # sequence chunk (query chunk = projection chunk)
NCH = S // CH    # 4 chunks
KO = D // 128    # contraction chunks for the projections
NJ = S // 128    # key tiles
ISQRT_DK = 1.0 / np.sqrt(DK)

_CACHE = {}

last_exec_time_ns = None
last_results = None


def _build_program():
    import concourse.mybir as mybir
    import concourse.tile as tile
    from concourse import bacc

    dt = mybir.dt
    F32 = dt.float32
    BF16 = dt.bfloat16
    AF = mybir.ActivationFunctionType

    nc = bacc.Bacc(None, target_bir_lowering=False, debug=True)

    xT = nc.dram_tensor("xT", [D, S], BF16, kind="ExternalInput")
    wqT = nc.dram_tensor("wqT", [D, E], BF16, kind="ExternalInput")
    wkT = nc.dram_tensor("wkT", [D, E], BF16, kind="ExternalInput")
    wvT = nc.dram_tensor("wvT", [D, E], BF16, kind="ExternalInput")
    woT = nc.dram_tensor("woT", [E, D], BF16, kind="ExternalInput")
    bq = nc.dram_tensor("bq", [HPC, DK], F32, kind="ExternalInput")
    bk = nc.dram_tensor("bk", [HPC, DK], F32, kind="ExternalInput")
    cc2 = nc.dram_tensor("cc2", [DK, S], BF16, kind="ExternalInput")
    sss = nc.dram_tensor("sss", [DK, S], BF16, kind="ExternalInput")
    mask = nc.dram_tensor("mask", [128, 128], BF16, kind="ExternalInput")
    ones = nc.dram_tensor("ones", [128, 128], BF16, kind="ExternalInput")
    out = nc.dram_tensor("out", [S, D], BF16, kind="ExternalOutput")

    with tile.TileContext(nc) as tc:
        with (
            tc.tile_pool(name="const", bufs=1) as cpool,
            tc.tile_pool(name="w", bufs=1) as wpool,
            tc.tile_pool(name="x", bufs=2) as xpool,
            tc.tile_pool(name="kres", bufs=1) as krpool,
            tc.tile_pool(name="vres", bufs=1) as vpool,
            tc.tile_pool(name="q", bufs=2) as qpool,
            tc.tile_pool(name="rope", bufs=3) as rpool,
            tc.tile_pool(name="p", bufs=4) as ppool,
            tc.tile_pool(name="pl", bufs=2) as plpool,
            tc.tile_pool(name="li", bufs=2) as lipool,
            tc.tile_pool(name="ao", bufs=2) as aopool,
            tc.tile_pool(name="ob", bufs=4) as obpool,
            tc.tile_pool(name="psum", bufs=1, space="PSUM") as pspool,
        ):
            # ---- constants (small; off the critical DMA path) ----
            bq_sb = cpool.tile([DK, HPC], F32, name="bq_sb")
            nc.gpsimd.dma_start(bq_sb[:], bq[:].rearrange("h d -> d h"))
            bk_sb = cpool.tile([DK, HPC], F32, name="bk_sb")
            nc.gpsimd.dma_start(bk_sb[:], bk[:].rearrange("h d -> d h"))
            cc2_sb = cpool.tile([DK, S], BF16, name="cc2_sb")
            nc.gpsimd.dma_start(cc2_sb[:], cc2[:])
            sss_sb = cpool.tile([DK, S], BF16, name="sss_sb")
            nc.gpsimd.dma_start(sss_sb[:], sss[:])
            mask_sb = cpool.tile([128, 128], BF16, name="mask_sb")
            nc.gpsimd.dma_start(mask_sb[:], mask[:])
            ones_sb = cpool.tile([128, 128], BF16, name="ones_sb")
            nc.gpsimd.dma_start(ones_sb[:], ones[:])

            # ---- bulk weights/x: need-ordered stream ----
            # first Q matmul chain consumes (wq[k], x0[k]) pairs in k order,
            # so interleave those at 512-row granularity; wk/wv/wo behind.
            wq_sb = wpool.tile([128, KO, E], BF16, name="wq_sb")
            wk_sb = wpool.tile([128, KO, E], BF16, name="wk_sb")
            wv_sb = wpool.tile([128, KO, E], BF16, name="wv_sb")
            wo_sb = wpool.tile([128, HPC, D], BF16, name="wo_sb")

            def load_xn(n):
                xn = xpool.tile([128, KO, CH], BF16, tag="xn", name=f"xn{n}")
                for g in range(4):
                    nc.sync.dma_start(
                        xn[:, g * 4 : (g + 1) * 4, :],
                        xT[
                            g * 512 : (g + 1) * 512,
                            n * CH : (n + 1) * CH,
                        ].rearrange("(ko p) s -> p ko s", p=128),
                    )
                return xn

            xn_next = xpool.tile([128, KO, CH], BF16, tag="xn", name="xn0")
            for k in range(KO):
                nc.sync.dma_start(
                    wq_sb[:, k, :], wqT[k * 128 : (k + 1) * 128, :]
                )
                nc.sync.dma_start(
                    xn_next[:, k, :], xT[k * 128 : (k + 1) * 128, 0:CH]
                )
            for wsb, wdram in ((wk_sb, wkT), (wv_sb, wvT)):
                for k in range(KO):
                    nc.sync.dma_start(
                        wsb[:, k, :], wdram[k * 128 : (k + 1) * 128, :]
                    )
            for ec in range(HPC):
                nc.sync.dma_start(
                    wo_sb[:, ec, :], woT[ec * 128 : (ec + 1) * 128, :]
                )

            kr = krpool.tile([128, HPC, S], BF16, name="kr")
            vres = vpool.tile([128, NJ, E], BF16, name="vres")

            def emit_c(ao_t, n_src, ii):
                # output projection for rows [n_src*CH + ii*128, +128)
                r0 = n_src * CH + ii * 128
                for fc in range(4):
                    pc = pspool.tile([128, 512], F32, tag="aps", bufs=3)
                    for ec in range(HPC):
                        nc.tensor.matmul(
                            pc[:],
                            ao_t[:, ec, ii * 128 : (ii + 1) * 128],
                            wo_sb[:, ec, fc * 512 : (fc + 1) * 512],
                            start=(ec == 0),
                            stop=(ec == HPC - 1),
                        )
                    ob = obpool.tile([128, 512], BF16, tag="ob")
                    if fc % 2 == 0:
                        nc.vector.tensor_copy(ob[:], pc[:])
                    else:
                        nc.scalar.activation(ob[:], pc[:], AF.Copy)
                    nc.scalar.dma_start(
                        out[r0 : r0 + 128, fc * 512 : (fc + 1) * 512], ob[:]
                    )

            ao_prev = None
            for n in range(NCH):
                nsl = slice(n * CH, (n + 1) * CH)
                xn = xn_next
                if n + 1 < NCH:
                    xn_next = load_xn(n + 1)

                # ---- stage A: project chunk n (+RoPE on Q/K) ----
                qc = qpool.tile([128, HPC, CH], BF16, tag="qc", name=f"qc{n}")

                def rope(pq, bsb, dst):
                    st0 = rpool.tile([128, CH], BF16, tag="st0")
                    nc.scalar.activation(
                        st0[:], pq[:], AF.Identity, bias=bsb
                    )
                    # RoPE: d-rows packed [even; odd] per head, so the
                    # rotate pair is partition r <-> r+64
                    sw = rpool.tile([128, CH], BF16, tag="sw")
                    nc.vector.tensor_copy(sw[0:64, :], st0[64:128, :])
                    nc.vector.tensor_copy(sw[64:128, :], st0[0:64, :])
                    rot = rpool.tile([128, CH], BF16, tag="rot")
                    nc.vector.tensor_mul(rot[:], st0[:], cc2_sb[:, nsl])
                    nc.vector.tensor_mul(sw[:], sw[:], sss_sb[:, nsl])
                    nc.vector.tensor_add(dst, rot[:], sw[:])

                def psum4():
                    # four simultaneously-live psum tiles borrowed from the
                    # aps(2) + ps(3) rings for chunk-0 k-outer chains
                    return [
                        pspool.tile(
                            [128, CH],
                            F32,
                            tag=("aps" if i < 2 else "ps"),
                            bufs=(3 if i < 2 else 3),
                            name=f"pk0_{i}",
                        )
                        for i in range(HPC)
                    ]

                if n == 0:
                    # k-outer on the first chunk: the PE consumes each
                    # (weight, x) 512KB DMA group as it lands instead of
                    # stalling on the full tensors
                    for wsb, bsb, is_q in (
                        (wq_sb, bq_sb, True),
                        (wk_sb, bk_sb, False),
                    ):
                        pqs = psum4()
                        for k in range(KO):
                            for m in range(HPC):
                                nc.tensor.matmul(
                                    pqs[m][:],
                                    wsb[:, k, m * DK : (m + 1) * DK],
                                    xn[:, k, :],
                                    start=(k == 0),
                                    stop=(k == KO - 1),
                                )
                        for m in range(HPC):
                            dst = qc[:, m, :] if is_q else kr[:, m, nsl]
                            rope(pqs[m], bsb[:, m : m + 1], dst)
                    pvs = psum4()
                    for k in range(KO):
                        for jj in range(CH // 128):
                            nc.tensor.matmul(
                                pvs[jj][:],
                                xn[:, k, jj * 128 : (jj + 1) * 128],
                                wv_sb[:, k, :],
                                start=(k == 0),
                                stop=(k == KO - 1),
                            )
                    for jj in range(CH // 128):
                        nc.scalar.activation(
                            vres[:, n * 4 + jj, :], pvs[jj][:], AF.Copy
                        )
                else:
                    for wsb, bsb, is_q in (
                        (wq_sb, bq_sb, True),
                        (wk_sb, bk_sb, False),
                    ):
                        for m in range(HPC):
                            pq = pspool.tile([128, CH], F32, tag="aps", bufs=3)
                            for k in range(KO):
                                nc.tensor.matmul(
                                    pq[:],
                                    wsb[:, k, m * DK : (m + 1) * DK],
                                    xn[:, k, :],
                                    start=(k == 0),
                                    stop=(k == KO - 1),
                                )
                            dst = qc[:, m, :] if is_q else kr[:, m, nsl]
                            rope(pq, bsb[:, m : m + 1], dst)
                    for jj in range(CH // 128):
                        pvp = pspool.tile([128, E], F32, tag="aps", bufs=3)
                        for k in range(KO):
                            nc.tensor.matmul(
                                pvp[:],
                                xn[:, k, jj * 128 : (jj + 1) * 128],
                                wv_sb[:, k, :],
                                start=(k == 0),
                                stop=(k == KO - 1),
                            )
                        nc.scalar.activation(
                            vres[:, n * 4 + jj, :], pvp[:], AF.Copy
                        )

                # ---- stage B: attention for query chunk n ----
                # ---- stage C (interleaved): out-proj for chunk n-1 ----
                ao_cur = aopool.tile(
                    [128, HPC, CH], BF16, tag="ao", name=f"ao{n}"
                )
                # a single full-width query window per chunk: narrower
                # windows make the ACT exp (whose ~370ns access-init is
                # per-instruction) the attention pacer
                wins = [(0, CH)]
                passes = [(q0, qn, m) for (q0, qn) in wins for m in range(HPC)]

                def pass_njc(q0, qn):
                    return 4 * n + (q0 + qn) // 128

                def score_exp(m, jc, q0, qn):
                    t = jc - (4 * n + q0 // 128)  # >=0 on the diagonal band
                    cs = 128 * t if t >= 0 else 0
                    ps = pspool.tile([128, CH], F32, tag="ps", bufs=3)
                    nc.tensor.matmul(
                        ps[:, cs:qn],
                        kr[:, m, jc * 128 : (jc + 1) * 128],
                        qc[:, m, q0 + cs : q0 + qn],
                        start=True,
                        stop=True,
                    )
                    p = ppool.tile([128, CH], BF16, tag="p")
                    nc.scalar.activation(
                        p[:, cs:qn], ps[:, cs:qn], AF.Exp,
                        scale=float(ISQRT_DK),
                    )
                    if t >= 0:
                        nc.vector.tensor_mul(
                            p[:, cs : cs + 128],
                            p[:, cs : cs + 128],
                            mask_sb[:],
                        )
                    return (p, jc, cs)

                # software pipeline: scores run up to three tiles ahead of
                # the P@V matmuls (and are pre-warmed across head
                # boundaries) so the ACT exp latency stays off the
                # tensor-engine path
                warm = []
                for pi, (q0, qn, m) in enumerate(passes):
                    njc = pass_njc(q0, qn)
                    po = pspool.tile([128, CH], F32, tag="po", bufs=2)
                    pl = plpool.tile([128, CH], BF16, tag="pl")

                    def emit_pv(p, jc, cs):
                        # denominator accumulate (DVE, bf16) + P@V (PE)
                        if jc == 0:
                            nc.vector.tensor_copy(pl[:, cs:qn], p[:, cs:qn])
                        else:
                            nc.vector.tensor_add(
                                pl[:, cs:qn], pl[:, cs:qn], p[:, cs:qn]
                            )
                        nc.tensor.matmul(
                            po[:, cs:qn],
                            vres[:, jc, m * DK : (m + 1) * DK],
                            p[:, cs:qn],
                            start=(jc == 0),
                            stop=(jc == njc - 1),
                        )

                    pending = warm
                    warm = []
                    for jc in range(len(pending), njc):
                        pending.append(score_exp(m, jc, q0, qn))
                        if len(pending) > 2:
                            emit_pv(*pending.pop(0))
                    for it in pending:
                        emit_pv(*it)

                    # collapse the 128 partial-denominator rows with one
                    # 128x128 ones matmul (borrowing a "ps" ring slot),
                    # then normalize
                    pstot = pspool.tile([128, CH], F32, tag="ps", bufs=3)
                    nc.tensor.matmul(
                        pstot[:, :qn], ones_sb[:], pl[:, :qn],
                        start=True, stop=True,
                    )
                    li = lipool.tile([128, CH], F32, tag="li")
                    nc.vector.reciprocal_approx_fast(li[:, :qn], pstot[:, :qn])

                    # pre-warm the next pass's first score tiles while the
                    # epilogue and out-projection run
                    if pi + 1 < len(passes):
                        nq0, nqn, nm = passes[pi + 1]
                        for jc in range(min(3, pass_njc(nq0, nqn))):
                            warm.append(score_exp(nm, jc, nq0, nqn))

                    nc.vector.tensor_mul(
                        ao_cur[:, m, q0 : q0 + qn], po[:, :qn], li[:, :qn]
                    )

                    if ao_prev is not None:
                        emit_c(ao_prev, n - 1, m)

                ao_prev = ao_cur

            for ii in range(4):
                emit_c(ao_prev, NCH - 1, ii)

    nc.compile()
    return nc


def _rope_tables():
    inv_freq = 1.0 / (10000.0 ** (np.arange(0, DK, 2, dtype=np.float64) / DK))
    pos = np.arange(S, dtype=np.float64)
    freqs = pos[:, None] * inv_freq[None, :]  # [S, DK/2]
    cos_t = np.cos(freqs).T.astype(np.float32)  # [64, S]
    sin_t = np.sin(freqs).T.astype(np.float32)
    cc2 = np.ascontiguousarray(np.concatenate([cos_t, cos_t], axis=0))
    sss = np.ascontiguousarray(np.concatenate([-sin_t, sin_t], axis=0))
    return cc2, sss


def _bf16(a):
    return np.ascontiguousarray(a.astype(ml_dtypes.bfloat16))


def kernel(
    x, wq_w, wq_b, wk_w, wk_b, wv_w, wv_b, wo_w, wo_b
) -> np.ndarray:
    global last_exec_time_ns, last_results
    from concourse.bass_utils import run_bass_kernel_spmd

    if "nc" not in _CACHE:
        _CACHE["nc"] = _build_program()
    nc = _CACHE["nc"]

    x = np.asarray(x, dtype=np.float32)
    wq_w = np.asarray(wq_w, dtype=np.float32)
    wk_w = np.asarray(wk_w, dtype=np.float32)
    wv_w = np.asarray(wv_w, dtype=np.float32)
    wo_w = np.asarray(wo_w, dtype=np.float32)
    wq_b = np.asarray(wq_b, dtype=np.float32)
    wk_b = np.asarray(wk_b, dtype=np.float32)
    wv_b = np.asarray(wv_b, dtype=np.float32)
    wo_b = np.asarray(wo_b, dtype=np.float32)

    cc2, sss = _rope_tables()
    r_idx = np.arange(128)[:, None]
    c_idx = np.arange(128)[None, :]
    mask_np = (r_idx <= c_idx).astype(np.float32)
    ones = np.ones((128, 128), dtype=np.float32)
    # within each head, pack d-rows as [even dims; odd dims]
    perm = np.concatenate([np.arange(0, DK, 2), np.arange(1, DK, 2)])

    xT_b = [_bf16(x[b].T) for b in range(B)]

    in_maps = []
    for c in range(N_CORES):
        b = c // (N_CORES // B)
        g = c % (N_CORES // B)
        es = g * E

        def pack_qk(w):
            rows = w[es : es + E]  # [E, D]
            blocks = [
                rows[h0 * DK : (h0 + 1) * DK][perm] for h0 in range(HPC)
            ]
            return _bf16(np.concatenate(blocks, axis=0).T)

        def pack_bias(bvec):
            sl = bvec[es : es + E].reshape(HPC, DK)
            return np.ascontiguousarray(sl[:, perm])

        in_maps.append(
            {
                "xT": xT_b[b],
                "wqT": pack_qk(wq_w),
                "wkT": pack_qk(wk_w),
                "wvT": _bf16(wv_w[es : es + E].T),
                "woT": _bf16(wo_w[:, es : es + E].T),
                "bq": pack_bias(wq_b),
                "bk": pack_bias(wk_b),
                "cc2": _bf16(cc2),
                "sss": _bf16(sss),
                "mask": _bf16(mask_np),
                "ones": _bf16(ones),
            }
        )

    trace = bool(os.environ.get("MHA_TRACE"))
    res = run_bass_kernel_spmd(
        nc, in_maps, list(range(N_CORES)), trace=trace
    )
    last_exec_time_ns = res.exec_time_ns
    last_results = res

    # host-side gather: sum partials per batch, add biases that commute
    # with attention (softmax rows sum to 1, so wv_b passes straight
    # through to the output projection)
    const_bias = wo_b + wo_w @ wv_b  # [D]
    out = np.empty((B, S, D), dtype=np.float32)
    gpb = N_CORES // B
    for b in range(B):
        acc = res.results[b * gpb]["out"].astype(np.float32)
        for c in range(b * gpb + 1, (b + 1) * gpb):
            acc += res.results[c]["out"].astype(np.float32)
        out[b] = acc + const_bias[None, :]
    return out


# revision 16
# speedup vs baseline: 1.0716x; 1.0076x over previous
"""Trainium2 Bass kernel for causal multi-head attention with RoPE.

Full-input contract: kernel(**inputs) takes the unsharded tensors and
returns the full [B, S, D] output. Internally the work is sharded over
8 NeuronCores: cores 0-3 compute batch 0, cores 4-7 batch 1; within a
batch group each core owns 4 of the 16 heads (tensor-parallel over
heads). Each core computes its partial output-projection contribution
[S, D]; the host sums the 4 partials per batch and adds the biases
that commute with attention (wo_b, and wv_b which passes through the
softmax untouched because attention weights sum to 1).

All matmul operands are bf16 (same 1 cycle/row PE rate as fp32r but
half the SBUF/DMA footprint and no small-free-dim penalty), with fp32
PSUM accumulation. The host converts inputs to bf16 before upload.

Every intermediate stays in SBUF. The three stages are interleaved per
512-query chunk n: project Q/K/V for chunk n (+RoPE), run causal
attention for query chunk n against keys 0..n, then emit the output
projection for chunk n-1. Softmax denominators are accumulated on the
DVE in bf16 and collapsed across partitions with a single 128x128 ones
matmul per (chunk, head) instead of a full row-sum matmul per score
tile, keeping that work off the PE.
"""

import os
import sys

sys.path.insert(0, "/opt/trn_rl_repo")

import numpy as np
import ml_dtypes

B = 2
S = 2048
D = 2048
H = 16
DK = 128
N_CORES = 8
HPC = 4          # heads per core
E = HPC * DK     # 512: per-core slice of the model dim
CH = 512         # sequence chunk (query chunk = projection chunk)
NCH = S // CH    # 4 chunks
KO = D // 128    # contraction chunks for the projections
NJ = S // 128    # key tiles
ISQRT_DK = 1.0 / np.sqrt(DK)

_CACHE = {}

last_exec_time_ns = None
last_results = None


def _build_program():
    import concourse.mybir as mybir
    import concourse.tile as tile
    from concourse import bacc

    dt = mybir.dt
    F32 = dt.float32
    BF16 = dt.bfloat16
    AF = mybir.ActivationFunctionType

    nc = bacc.Bacc(None, target_bir_lowering=False, debug=True)

    # inputs are host-packed into the exact SBUF layouts so every DMA
    # descriptor is 128 contiguous per-partition segments (descriptor
    # issue on the sequencer costs ~2.8ns per segment)
    xP = nc.dram_tensor("xP", [NCH, 128, KO, CH], BF16, kind="ExternalInput")
    wqT = nc.dram_tensor("wqT", [128, KO, E], BF16, kind="ExternalInput")
    wkT = nc.dram_tensor("wkT", [128, KO, E], BF16, kind="ExternalInput")
    wvT = nc.dram_tensor("wvT", [128, KO, E], BF16, kind="ExternalInput")
    woT = nc.dram_tensor("woT", [128, HPC, D], BF16, kind="ExternalInput")
    bq = nc.dram_tensor("bq", [HPC, DK], F32, kind="ExternalInput")
    bk = nc.dram_tensor("bk", [HPC, DK], F32, kind="ExternalInput")
    cc2 = nc.dram_tensor("cc2", [DK, S], BF16, kind="ExternalInput")
    sss = nc.dram_tensor("sss", [DK, S], BF16, kind="ExternalInput")
    mask = nc.dram_tensor("mask", [128, 128], BF16, kind="ExternalInput")
    ones = nc.dram_tensor("ones", [128, 128], BF16, kind="ExternalInput")
    out = nc.dram_tensor("out", [S, D], BF16, kind="ExternalOutput")

    with tile.TileContext(nc) as tc:
        with (
            tc.tile_pool(name="const", bufs=1) as cpool,
            tc.tile_pool(name="w", bufs=1) as wpool,
            tc.tile_pool(name="x", bufs=2) as xpool,
            tc.tile_pool(name="kres", bufs=1) as krpool,
            tc.tile_pool(name="vres", bufs=1) as vpool,
            tc.tile_pool(name="q", bufs=2) as qpool,
            tc.tile_pool(name="rope", bufs=3) as rpool,
            tc.tile_pool(name="p", bufs=4) as ppool,
            tc.tile_pool(name="pl", bufs=2) as plpool,
            tc.tile_pool(name="li", bufs=2) as lipool,
            tc.tile_pool(name="ao", bufs=2) as aopool,
            tc.tile_pool(name="ob", bufs=4) as obpool,
            tc.tile_pool(name="psum", bufs=1, space="PSUM") as pspool,
        ):
            # ---- constants (small; off the critical DMA path) ----
            bq_sb = cpool.tile([DK, HPC], F32, name="bq_sb")
            nc.gpsimd.dma_start(bq_sb[:], bq[:].rearrange("h d -> d h"))
            bk_sb = cpool.tile([DK, HPC], F32, name="bk_sb")
            nc.gpsimd.dma_start(bk_sb[:], bk[:].rearrange("h d -> d h"))
            cc2_sb = cpool.tile([DK, S], BF16, name="cc2_sb")
            nc.gpsimd.dma_start(cc2_sb[:], cc2[:])
            sss_sb = cpool.tile([DK, S], BF16, name="sss_sb")
            nc.gpsimd.dma_start(sss_sb[:], sss[:])
            mask_sb = cpool.tile([128, 128], BF16, name="mask_sb")
            nc.gpsimd.dma_start(mask_sb[:], mask[:])
            ones_sb = cpool.tile([128, 128], BF16, name="ones_sb")
            nc.gpsimd.dma_start(ones_sb[:], ones[:])

            # ---- bulk weights/x: need-ordered stream ----
            # first Q matmul chain consumes (wq[k], x0[k]) pairs in k order,
            # so interleave those at 512-row granularity; wk/wv/wo behind.
            wq_sb = wpool.tile([128, KO, E], BF16, name="wq_sb")
            wk_sb = wpool.tile([128, KO, E], BF16, name="wk_sb")
            wv_sb = wpool.tile([128, KO, E], BF16, name="wv_sb")
            wo_sb = wpool.tile([128, HPC, D], BF16, name="wo_sb")

            def load_xn(n):
                xn = xpool.tile([128, KO, CH], BF16, tag="xn", name=f"xn{n}")
                nc.sync.dma_start(xn[:], xP[n])
                return xn

            xn_next = xpool.tile([128, KO, CH], BF16, tag="xn", name="xn0")
            for g in range(KO // 4):
                gs = slice(4 * g, 4 * g + 4)
                nc.sync.dma_start(wq_sb[:, gs, :], wqT[:, gs, :])
                nc.sync.dma_start(xn_next[:, gs, :], xP[0][:, gs, :])
            for wsb, wdram in ((wk_sb, wkT), (wv_sb, wvT)):
                for g in range(KO // 4):
                    gs = slice(4 * g, 4 * g + 4)
                    nc.sync.dma_start(wsb[:, gs, :], wdram[:, gs, :])
            nc.sync.dma_start(wo_sb[:], woT[:])

            kr = krpool.tile([128, HPC, S], BF16, name="kr")
            vres = vpool.tile([128, NJ, E], BF16, name="vres")

            def emit_c(ao_t, n_src, ii):
                # output projection for rows [n_src*CH + ii*128, +128)
                r0 = n_src * CH + ii * 128
                for fc in range(4):
                    pc = pspool.tile([128, 512], F32, tag="aps", bufs=3)
                    for ec in range(HPC):
                        nc.tensor.matmul(
                            pc[:],
                            ao_t[:, ec, ii * 128 : (ii + 1) * 128],
                            wo_sb[:, ec, fc * 512 : (fc + 1) * 512],
                            start=(ec == 0),
                            stop=(ec == HPC - 1),
                        )
                    ob = obpool.tile([128, 512], BF16, tag="ob")
                    if fc % 2 == 0:
                        nc.vector.tensor_copy(ob[:], pc[:])
                    else:
                        nc.scalar.activation(ob[:], pc[:], AF.Copy)
                    nc.scalar.dma_start(
                        out[r0 : r0 + 128, fc * 512 : (fc + 1) * 512], ob[:]
                    )

            ao_prev = None
            for n in range(NCH):
                nsl = slice(n * CH, (n + 1) * CH)
                xn = xn_next
                if n + 1 < NCH:
                    xn_next = load_xn(n + 1)

                # ---- stage A: project chunk n (+RoPE on Q/K) ----
                qc = qpool.tile([128, HPC, CH], BF16, tag="qc", name=f"qc{n}")

                def rope(pq, bsb, dst):
                    st0 = rpool.tile([128, CH], BF16, tag="st0")
                    nc.scalar.activation(
                        st0[:], pq[:], AF.Identity, bias=bsb
                    )
                    # RoPE: d-rows packed [even; odd] per head, so the
                    # rotate pair is partition r <-> r+64
                    sw = rpool.tile([128, CH], BF16, tag="sw")
                    nc.vector.tensor_copy(sw[0:64, :], st0[64:128, :])
                    nc.vector.tensor_copy(sw[64:128, :], st0[0:64, :])
                    rot = rpool.tile([128, CH], BF16, tag="rot")
                    nc.vector.tensor_mul(rot[:], st0[:], cc2_sb[:, nsl])
                    nc.vector.tensor_mul(sw[:], sw[:], sss_sb[:, nsl])
                    nc.vector.tensor_add(dst, rot[:], sw[:])

                def psum4():
                    # four simultaneously-live psum tiles borrowed from the
                    # aps(2) + ps(3) rings for chunk-0 k-outer chains
                    return [
                        pspool.tile(
                            [128, CH],
                            F32,
                            tag=("aps" if i < 2 else "ps"),
                            bufs=(3 if i < 2 else 3),
                            name=f"pk0_{i}",
                        )
                        for i in range(HPC)
                    ]

                if n == 0:
                    # k-outer on the first chunk: the PE consumes each
                    # (weight, x) 512KB DMA group as it lands instead of
                    # stalling on the full tensors
                    for wsb, bsb, is_q in (
                        (wq_sb, bq_sb, True),
                        (wk_sb, bk_sb, False),
                    ):
                        pqs = psum4()
                        for k in range(KO):
                            for m in range(HPC):
                                nc.tensor.matmul(
                                    pqs[m][:],
                                    wsb[:, k, m * DK : (m + 1) * DK],
                                    xn[:, k, :],
                                    start=(k == 0),
                                    stop=(k == KO - 1),
                                )
                        for m in range(HPC):
                            dst = qc[:, m, :] if is_q else kr[:, m, nsl]
                            rope(pqs[m], bsb[:, m : m + 1], dst)
                    pvs = psum4()
                    for k in range(KO):
                        for jj in range(CH // 128):
                            nc.tensor.matmul(
                                pvs[jj][:],
                                xn[:, k, jj * 128 : (jj + 1) * 128],
                                wv_sb[:, k, :],
                                start=(k == 0),
                                stop=(k == KO - 1),
                            )
                    for jj in range(CH // 128):
                        nc.scalar.activation(
                            vres[:, n * 4 + jj, :], pvs[jj][:], AF.Copy
                        )
                else:
                    for wsb, bsb, is_q in (
                        (wq_sb, bq_sb, True),
                        (wk_sb, bk_sb, False),
                    ):
                        for m in range(HPC):
                            pq = pspool.tile([128, CH], F32, tag="aps", bufs=3)
                            for k in range(KO):
                                nc.tensor.matmul(
                                    pq[:],
                                    wsb[:, k, m * DK : (m + 1) * DK],
                                    xn[:, k, :],
                                    start=(k == 0),
                                    stop=(k == KO - 1),
                                )
                            dst = qc[:, m, :] if is_q else kr[:, m, nsl]
                            rope(pq, bsb[:, m : m + 1], dst)
                    for jj in range(CH // 128):
                        pvp = pspool.tile([128, E], F32, tag="aps", bufs=3)
                        for k in range(KO):
                            nc.tensor.matmul(
                                pvp[:],
                                xn[:, k, jj * 128 : (jj + 1) * 128],
                                wv_sb[:, k, :],
                                start=(k == 0),
                                stop=(k == KO - 1),
                            )
                        nc.scalar.activation(
                            vres[:, n * 4 + jj, :], pvp[:], AF.Copy
                        )

                # ---- stage B: attention for query chunk n ----
                # ---- stage C (interleaved): out-proj for chunk n-1 ----
                ao_cur = aopool.tile(
                    [128, HPC, CH], BF16, tag="ao", name=f"ao{n}"
                )
                # a single full-width query window per chunk: narrower
                # windows make the ACT exp (whose ~370ns access-init is
                # per-instruction) the attention pacer
                wins = [(0, CH)]
                passes = [(q0, qn, m) for (q0, qn) in wins for m in range(HPC)]

                def pass_njc(q0, qn):
                    return 4 * n + (q0 + qn) // 128

                def score_exp(m, jc, q0, qn):
                    t = jc - (4 * n + q0 // 128)  # >=0 on the diagonal band
                    cs = 128 * t if t >= 0 else 0
                    ps = pspool.tile([128, CH], F32, tag="ps", bufs=3)
                    nc.tensor.matmul(
                        ps[:, cs:qn],
                        kr[:, m, jc * 128 : (jc + 1) * 128],
                        qc[:, m, q0 + cs : q0 + qn],
                        start=True,
                        stop=True,
                    )
                    p = ppool.tile([128, CH], BF16, tag="p")
                    nc.scalar.activation(
                        p[:, cs:qn], ps[:, cs:qn], AF.Exp,
                        scale=float(ISQRT_DK),
                    )
                    if t >= 0:
                        nc.vector.tensor_mul(
                            p[:, cs : cs + 128],
                            p[:, cs : cs + 128],
                            mask_sb[:],
                        )
                    return (p, jc, cs)

                # software pipeline: scores run up to three tiles ahead of
                # the P@V matmuls (and are pre-warmed across head
                # boundaries) so the ACT exp latency stays off the
                # tensor-engine path
                warm = []
                for pi, (q0, qn, m) in enumerate(passes):
                    njc = pass_njc(q0, qn)
                    po = pspool.tile([128, CH], F32, tag="po", bufs=2)
                    pl = plpool.tile([128, CH], BF16, tag="pl")

                    def emit_pv(p, jc, cs):
                        # denominator accumulate (DVE, bf16) + P@V (PE)
                        if jc == 0:
                            nc.vector.tensor_copy(pl[:, cs:qn], p[:, cs:qn])
                        else:
                            nc.vector.tensor_add(
                                pl[:, cs:qn], pl[:, cs:qn], p[:, cs:qn]
                            )
                        nc.tensor.matmul(
                            po[:, cs:qn],
                            vres[:, jc, m * DK : (m + 1) * DK],
                            p[:, cs:qn],
                            start=(jc == 0),
                            stop=(jc == njc - 1),
                        )

                    pending = warm
                    warm = []
                    for jc in range(len(pending), njc):
                        pending.append(score_exp(m, jc, q0, qn))
                        if len(pending) > 2:
                            emit_pv(*pending.pop(0))
                    for it in pending:
                        emit_pv(*it)

                    # pre-warm the next pass's first score tiles: they keep
                    # the PE busy while the last denominator adds drain on
                    # the DVE, and cover the exp latency of the next pass
                    if pi + 1 < len(passes):
                        nq0, nqn, nm = passes[pi + 1]
                        for jc in range(min(3, pass_njc(nq0, nqn))):
                            warm.append(score_exp(nm, jc, nq0, nqn))

                    # collapse the 128 partial-denominator rows with one
                    # 128x128 ones matmul (borrowing a "ps" ring slot),
                    # then normalize
                    pstot = pspool.tile([128, CH], F32, tag="ps", bufs=3)
                    nc.tensor.matmul(
                        pstot[:, :qn], ones_sb[:], pl[:, :qn],
                        start=True, stop=True,
                    )
                    li = lipool.tile([128, CH], F32, tag="li")
                    nc.vector.reciprocal_approx_fast(li[:, :qn], pstot[:, :qn])
                    nc.vector.tensor_mul(
                        ao_cur[:, m, q0 : q0 + qn], po[:, :qn], li[:, :qn]
                    )

                    if ao_prev is not None:
                        emit_c(ao_prev, n - 1, m)

                ao_prev = ao_cur

            for ii in range(4):
                emit_c(ao_prev, NCH - 1, ii)

    nc.compile()
    return nc


def _rope_tables():
    inv_freq = 1.0 / (10000.0 ** (np.arange(0, DK, 2, dtype=np.float64) / DK))
    pos = np.arange(S, dtype=np.float64)
    freqs = pos[:, None] * inv_freq[None, :]  # [S, DK/2]
    cos_t = np.cos(freqs).T.astype(np.float32)  # [64, S]
    sin_t = np.sin(freqs).T.astype(np.float32)
    cc2 = np.ascontiguousarray(np.concatenate([cos_t, cos_t], axis=0))
    sss = np.ascontiguousarray(np.concatenate([-sin_t, sin_t], axis=0))
    return cc2, sss


def _bf16(a):
    return np.ascontiguousarray(a.astype(ml_dtypes.bfloat16))


def kernel(
    x, wq_w, wq_b, wk_w, wk_b, wv_w, wv_b, wo_w, wo_b
) -> np.ndarray:
    global last_exec_time_ns, last_results
    from concourse.bass_utils import run_bass_kernel_spmd

    if "nc" not in _CACHE:
        _CACHE["nc"] = _build_program()
    nc = _CACHE["nc"]

    x = np.asarray(x, dtype=np.float32)
    wq_w = np.asarray(wq_w, dtype=np.float32)
    wk_w = np.asarray(wk_w, dtype=np.float32)
    wv_w = np.asarray(wv_w, dtype=np.float32)
    wo_w = np.asarray(wo_w, dtype=np.float32)
    wq_b = np.asarray(wq_b, dtype=np.float32)
    wk_b = np.asarray(wk_b, dtype=np.float32)
    wv_b = np.asarray(wv_b, dtype=np.float32)
    wo_b = np.asarray(wo_b, dtype=np.float32)

    cc2, sss = _rope_tables()
    r_idx = np.arange(128)[:, None]
    c_idx = np.arange(128)[None, :]
    mask_np = (r_idx <= c_idx).astype(np.float32)
    ones = np.ones((128, 128), dtype=np.float32)
    # within each head, pack d-rows as [even dims; odd dims]
    perm = np.concatenate([np.arange(0, DK, 2), np.arange(1, DK, 2)])

    # pack to the on-chip layouts: [p, ko, cols] with p the SBUF partition
    def sb_pack(wT):  # [D, cols] -> [128, KO, cols]
        return wT.reshape(KO, 128, wT.shape[1]).transpose(1, 0, 2)

    xP_b = [
        _bf16(
            x[b].T.reshape(KO, 128, NCH, CH).transpose(2, 1, 0, 3)
        )
        for b in range(B)
    ]

    in_maps = []
    for c in range(N_CORES):
        b = c // (N_CORES // B)
        g = c % (N_CORES // B)
        es = g * E

        def pack_qk(w):
            rows = w[es : es + E]  # [E, D]
            blocks = [
                rows[h0 * DK : (h0 + 1) * DK][perm] for h0 in range(HPC)
            ]
            return _bf16(sb_pack(np.concatenate(blocks, axis=0).T))

        def pack_bias(bvec):
            sl = bvec[es : es + E].reshape(HPC, DK)
            return np.ascontiguousarray(sl[:, perm])

        in_maps.append(
            {
                "xP": xP_b[b],
                "wqT": pack_qk(wq_w),
                "wkT": pack_qk(wk_w),
                "wvT": _bf16(sb_pack(wv_w[es : es + E].T)),
                "woT": _bf16(
                    wo_w[:, es : es + E].T.reshape(HPC, 128, D).transpose(
                        1, 0, 2
                    )
                ),
                "bq": pack_bias(wq_b),
                "bk": pack_bias(wk_b),
                "cc2": _bf16(cc2),
                "sss": _bf16(sss),
                "mask": _bf16(mask_np),
                "ones": _bf16(ones),
            }
        )

    trace = bool(os.environ.get("MHA_TRACE"))
    res = run_bass_kernel_spmd(
        nc, in_maps, list(range(N_CORES)), trace=trace
    )
    last_exec_time_ns = res.exec_time_ns
    last_results = res

    # host-side gather: sum partials per batch, add biases that commute
    # with attention (softmax rows sum to 1, so wv_b passes straight
    # through to the output projection)
    const_bias = wo_b + wo_w @ wv_b  # [D]
    out = np.empty((B, S, D), dtype=np.float32)
    gpb = N_CORES // B
    for b in range(B):
        acc = res.results[b * gpb]["out"].astype(np.float32)
        for c in range(b * gpb + 1, (b + 1) * gpb):
            acc += res.results[c]["out"].astype(np.float32)
        out[b] = acc + const_bias[None, :]
    return out
